# revision 64
# baseline (speedup 1.0000x reference)
"""Bilateral filter (3x3, sigma=0.8) Trainium2 Bass kernel. v9

Sharding: fully data-parallel over the fused batch B*V = 8 -> one
(C=3,H=512,W=512) image per NeuronCore, 8 cores.

Per-core layout: H=512 rows split 4 rows/partition over 128 partitions.
Each partition holds 5 rows (4 data rows + 1 halo row below) x 520 cols
(2 left pad, 512 data, 6 right pad) flattened in the free dimension, so
every 3x3 tap at +e is a constant flat offset.  The host bakes all pad
zeros into the DMA'd planes, so no on-chip memsets gate the input DMA.

Math (unnormalized weights; per-pixel wd/wc normalizations cancel in
num/den; the 1e-7 eps is dropped, |effect| ~1e-4):
  For e in {(0,1),(1,0),(1,1),(1,-1)} (pair symmetry covers -e):
    g_e  = DErf(sqrt(S)(d(+e)-d))            # (2/sqrt(pi)) exp(-S dd^2)
    G_e  = sum_c DErf(sqrt(S)(c_c(+e)-c_c))
    F_e  = g_e * G_e                         # ws_e folded into PE weights
    den += ws_e*(F_e*m(+e) @0 + F_e*m @-e)   (+ center WCEN*m)
    num_c += ws_e*(F_e*mc_c(+e) @0 + F_e*mc_c @-e) (+ center WCEN*mc_c)
  with mc_c = m*c_c precomputed once.  The DErf constant (2/sqrt(pi))^2
  is uniform across taps once the center is scaled by 4/pi, so it
  cancels in num/den.

Engine split (the DVE is the bottleneck; the HAM clock governor is a
POWER budget, so junk/offload work on other engines slows everything —
GpSimd compute measured ~20% global clock loss):
 - DVE: diffs/products, all fused where flat offsets allow: the three
   ef-consecutive dirs (519,520,521) share one op via overlapping-window
   APs; {A,B}-side product pairs share one op via a window-pair AP with
   the F field broadcast (stride-0); all verified to keep DVE 2x mode.
 - ACT: DErf/Ln/Exp/PSUM copies.
 - PE: G channel-sum (scaled-identity matmuls into PSUM) + all 9-tap
   accumulations of den/num_c; per-row FD-512 matmuls (PSUM bank cap);
   -e taps crossing a partition boundary use shift matrices
   sh=eye(k=1)*ws, deferred per accumulation for Ldweights dedup.
Finals run in 2-row halves (PSUM copy / 1/den multiply / store DMA
pipeline; the last channel's second half in 1-row quarters) to shorten
the serial tail.
"""

import math
import numpy as np
import sys

if "/opt/trn_rl_repo" not in sys.path:
    sys.path.insert(0, "/opt/trn_rl_repo")

import concourse.bass as bass
import concourse.tile as tile
from concourse import mybir
from concourse.bass_utils import run_bass_kernel_spmd

# ---- problem constants (hardcoded per spec) ----
B, V, C, H, W = 2, 4, 3, 512, 512
N_CORES = 8
KS = 3
SIG = 0.3 * ((KS - 1) * 0.5 - 1) + 0.8           # 0.8
S = 1.0 / (2.0 * SIG * SIG)                       # 0.78125

# spatial gaussian, normalized
_xs = np.arange(KS, dtype=np.float64)
_gx, _gy = np.meshgrid(_xs, _xs, indexing="xy")
_w = np.exp(-(((_gx - 1) ** 2 + (_gy - 1) ** 2)) * S)
_w = _w / _w.sum()
W0 = float(_w[1, 1])   # center
W1 = float(_w[0, 1])   # edge-adjacent
W2 = float(_w[0, 0])   # diagonal

# layout constants
R = 4                  # data rows per partition
W2C = 520              # row stride (2 left pad + 512 data + 6 right pad)
NROW = 5               # rows per partition incl. bottom halo
FLAT = NROW * W2C      # 2600
ALLOC = FLAT + 24      # slack so reads at +521 from flat 2079 stay in-bounds
PROD = R * W2C         # 2080: field/product grid (4 rows, all cols)
COL0 = 2               # first data col

# (er, ec, flat offset, spatial weight index)
ES = [(0, 1, 1, 0), (1, 0, W2C, 0), (1, 1, W2C + 1, 1), (1, -1, W2C - 1, 1)]
SQS = math.sqrt(S)          # DErf(SQS*x) = 2/sqrt(pi) * exp(-S x^2)
PHI2 = 4.0 / math.pi        # (2/sqrt(pi))^2, folded into the center tap
WCEN = 3.0 * W0 * PHI2

# ---- tuning knobs ----
USE_PE_G = True             # G channel-sum via PE matmuls + ACT copy
N_WARM = 0                  # PE warm-up matmuls (fills pre-field idle)

# dirs sorted by flat offset ef (519, 520, 521 are consecutive, so the
# phase-B products for those three dirs fuse into one DVE op via an
# overlapping-window AP); SLOT maps dir index -> slot in the Fall tile
SLOT = {0: 0, 3: 1, 1: 2, 2: 3}   # dir -> slot in the Fall tile


def _window_ap(ap2d, off, count, length, step=1):
    """[p, count, length] view of a flat [p, N] AP with overlapping
    windows starting at off, off+step, ..., off+(count-1)*step."""
    w = ap2d[:, off:off + length].unsqueeze(1).copy()
    v = w.ap
    v[1] = [step, count]
    w.ap = v
    return w

F16 = mybir.dt.float16
F32 = mybir.dt.float32
AF = mybir.ActivationFunctionType
ALU = mybir.AluOpType

# weight-matrix slots in the idents tile
ID_PLAIN, ID_W1, ID_W2, ID_CEN, ID_SH1, ID_SH2, ID_NEG = range(7)


# ---- walrus single-wait workaround ----------------------------------------
# This container's walrus accepts only ONE sync_info.on_wait per instruction;
# Tile emits multi-wait instructions. Hoist all but the last wait onto
# injected single-wait NoOps just before the original.
import orjson as _orjson

_SCRATCH = "wsplit_scratch"


def _mk_nop(name, engine, wait):
    return {"name": name, "engine": engine, "ins": [], "outs": [],
            "opcode": "NoOp",
            "sync_info": {"on_wait": [wait], "on_update": []}}


def _ldw_sig(ins):
    aps = ins.get("ins") or []
    if not aps:
        return None
    a = aps[0]
    return (a.get("memref"), a.get("offset"), str(a.get("ap")), a.get("dtype"))


def _dedup_ldweights(m):
    """NoOp-ify PE Ldweights whose weights are already loaded (same static
    source AP as the previous Ldweights, sourced from the idents tile).
    Sync info is preserved on the NoOp."""
    for f in m.get("functions", []):
        for bb in f.get("blocks", []):
            last = None
            for ins in bb.get("instructions", []):
                if ins.get("opcode") != "Ldweights":
                    continue
                sig = _ldw_sig(ins)
                if (sig is not None and sig == last
                        and sig[0] and "idents" in sig[0]):
                    ins["opcode"] = "NoOp"
                    ins["ins"] = []
                    ins["outs"] = []
                else:
                    last = sig
    return m


def _split_multiwaits(bir_bytes):
    m = _orjson.loads(bir_bytes)
    _dedup_ldweights(m)
    for f in m.get("functions", []):
        for bb in f.get("blocks", []):
            out = []
            for ins in bb.get("instructions", []):
                si = ins.get("sync_info")
                waits = (si or {}).get("on_wait") or []
                if len(waits) > 1:
                    for k, w in enumerate(waits[:-1]):
                        nm = f"{ins['name']}-wsplit{k}"
                        out.append(_mk_nop(nm, ins["engine"], w))
                    si["on_wait"] = [waits[-1]]
                out.append(ins)
            bb["instructions"] = out
    return _orjson.dumps(m)


_BUILD_CACHE = {}


def _build_nc():
    nc = bass.Bass()
    x_in = nc.declare_dram_parameter("x", [5, 128, ALLOC], F16, isOutput=False)
    id_in = nc.declare_dram_parameter("ident", [7, 128, 128], F16, isOutput=False)
    o_out = nc.declare_dram_parameter("out", [C, H, W], F16, isOutput=True)
    nc.dram_tensor(_SCRATCH, [4], F32)

    with tile.TileContext(nc) as tc:
        _emit(nc, tc, x_in, id_in, o_out)

    orig_to_json = nc.to_json_bytes
    nc.to_json_bytes = lambda: _split_multiwaits(orig_to_json())
    return nc


def _emit(nc, tc, x_in, id_in, o_out):
    from contextlib import ExitStack
    ctx = ExitStack()
    with ctx:
        persist = ctx.enter_context(tc.tile_pool(name="persist", bufs=1))
        tdp = ctx.enter_context(tc.tile_pool(name="tdp", bufs=2))
        tcap = ctx.enter_context(tc.tile_pool(name="tcap", bufs=2))
        fp = ctx.enter_context(tc.tile_pool(name="fp", bufs=2))
        fmp = ctx.enter_context(tc.tile_pool(name="fmp", bufs=2))
        yz_p = ctx.enter_context(tc.tile_pool(name="yz", bufs=4))
        fin_p = ctx.enter_context(tc.tile_pool(name="fin", bufs=2))
        psum_p = ctx.enter_context(
            tc.tile_pool(name="psum", bufs=1, space=bass.MemorySpace.PSUM)
        )

        # ---- persistent fp16 planes / fields ----
        d16 = persist.tile([128, ALLOC], F16, tag="d16", name="d16")
        m16 = persist.tile([128, ALLOC], F16, tag="m16", name="m16")
        c16all = persist.tile([128, C, ALLOC], F16, tag="c16all", name="c16all")
        c16 = [c16all[:, i, :] for i in range(C)]
        mc3 = persist.tile([128, C, ALLOC], F16, tag="mc3", name="mc3")
        idents = persist.tile([128, 7, 128], F16, tag="idents", name="idents")
        wmat = [idents[:, j, :] for j in range(7)]
        Fall = persist.tile([128, 4, PROD], F16, tag="Fall", name="Fall")
        Ft = [Fall[:, SLOT[i], :] for i in range(4)]
        r16 = persist.tile([128, R, W], F16, tag="r16", name="r16")
        lden = persist.tile([128, R, W], F32, tag="lden", name="lden")

        # ---- load planes (fully pre-padded host-side; no memsets) ----
        # order: d first (fields), then c0..c2 (tca), m, idents early for
        # PE warm-up.  Two HWDGE queues: sync + scalar.
        nc.scalar.dma_start(idents[:], id_in.rearrange("j p c -> p j c"))
        nc.sync.dma_start(d16[:], x_in[0])
        nc.scalar.dma_start(c16all[:, 0, :], x_in[1])
        nc.sync.dma_start(c16all[:, 1, :], x_in[2])
        nc.scalar.dma_start(c16all[:, 2, :], x_in[3])
        nc.sync.dma_start(m16[:], x_in[4])

        den = psum_p.tile([128, R, W], F32, tag="acc", name="den", bufs=2)

        # NOTE: the HAM clock governor enforces a POWER budget — dummy
        # "heater" work makes the whole kernel clock LOWER (measured
        # +30us).  Keep extra activity minimal; a short PE warm-up only.
        for k in range(N_WARM):
            nc.tensor.matmul(
                den[:, 0, 0:W], wmat[ID_PLAIN], idents[:, 0:4, :],
                start=True, stop=True, skip_group_check=True,
            )

        # mc_c = m * c_c on the full 5-row grid (halos included); one
        # fused op over all 3 channels with m broadcast (stride-0 dim)
        nc.vector.tensor_mul(
            mc3[:, :, :], c16all[:, :, :],
            m16[:].unsqueeze(1).broadcast_to([128, C, ALLOC]))

        def mm(acc, wi, rhs_flat, off, row, start=False, stop=False,
               n=W, ocol=0):
            nc.tensor.matmul(
                acc[:, row, ocol:ocol + n], wmat[wi],
                rhs_flat[:, off:off + n],
                start=start, stop=stop,
            )

        def accum_dir(acc, i, a_t, b_t, first, defer=None, a_only=False):
            """acc += ws_e*(A-term at 0) + ws_e*(B-term at -e) for dir i.
            B windows skip the boundary column (where the halo tap is
            zero); the cross-partition row-0 B tap uses a shift matrix.
            If `defer` is a list the shift matmul is queued there instead
            (flush with flush_sh) so consecutive dirs keep the same PE
            weights loaded (Ldweights dedup).  a_only skips the B block
            (used when a_t already holds the A+B combined plane)."""
            er, ec, ef, iw = ES[i]
            widn = ID_W1 if iw == 0 else ID_W2
            wsh = ID_SH1 if iw == 0 else ID_SH2
            for r in range(R):
                mm(acc, widn, a_t, r * W2C + COL0, r, start=first)
            if a_only:
                return
            n = W - abs(ec)
            ocol = max(0, ec)
            icol = COL0 + max(0, -ec)
            if er == 0:
                for r in range(R):
                    mm(acc, widn, b_t, r * W2C + icol, r, n=n, ocol=ocol)
            else:
                for r in range(1, R):
                    mm(acc, widn, b_t, (r - 1) * W2C + icol, r, n=n, ocol=ocol)
                if defer is None:
                    mm(acc, wsh, b_t, 3 * W2C + icol, 0, n=n, ocol=ocol)
                else:
                    defer.append((wsh, b_t, 3 * W2C + icol, n, ocol))

        def flush_sh(acc, deferred):
            deferred.sort(key=lambda t: t[0])
            for wsh, b_t, off, n, ocol in deferred:
                mm(acc, wsh, b_t, off, 0, n=n, ocol=ocol)
            deferred.clear()

        def accum_cen(acc, cen_t, stop=True):
            for r in range(R):
                mm(acc, ID_CEN, cen_t, r * W2C + COL0, r, stop=(stop and r == R - 1))

        # ---- phase A: per-dir fields + den accumulation; ch0 products ----
        # USE_PE_G: G channel-sum runs on the PE into a PSUM tile (gps),
        # copied back to fp16 SBUF by the scalar engine; this takes 8
        # tensor_adds off the DVE but occupies 4 PSUM banks, deferring
        # num0 accumulation to phase B.  Otherwise G sums on the DVE and
        # den+num0 accumulate interleaved in phase A (balanced PE phases).
        if USE_PE_G:
            gps = psum_p.tile([128, R, W], F32, tag="acc", name="gps",
                              bufs=2)
        else:
            num0 = psum_p.tile([128, R, W], F32, tag="acc", name="num0",
                               bufs=2)

        # td fields: dir0 alone (earliest), dirs 3,1,2 fused in one op via
        # the overlapping-window AP (ef = 519,520,521) with d16 broadcast
        td0 = tdp.tile([128, PROD], F16, tag="td", name="td0", bufs=1)
        nc.vector.tensor_sub(td0[:], d16[:, 1:PROD + 1], d16[:, 0:PROD])
        nc.scalar.activation(td0[:], td0[:], AF.Derivative_Erf, scale=SQS)
        tdf = tdp.tile([128, 3, PROD], F16, tag="tdf", name="tdf", bufs=1)
        nc.vector.tensor_sub(
            tdf[:], _window_ap(d16[:], W2C - 1, 3, PROD),
            d16[:, 0:PROD].unsqueeze(1).broadcast_to([128, 3, PROD]))
        nc.scalar.activation(tdf[:], tdf[:], AF.Derivative_Erf, scale=SQS)

        # zero both G ring slots once: per-dir G copies only write data
        # columns, so pads stay 0 forever; otherwise F = td*G inherits
        # NaN pads from uninitialized SBUF and the phase-B s0 combine
        # would read them through Z's left pad column
        for _ in range(2):
            gz = fp.tile([128, PROD], F16, tag="G", name="gz")
            nc.vector.memset(gz[:], 0.0)

        yz0 = []
        for i, (er, ec, ef, iw) in enumerate(ES):
            td = td0[:] if i == 0 else tdf[:, SLOT[i] - 1, :]

            tca = tcap.tile([128, C, PROD], F16, tag="tca", name="tca")
            G = fp.tile([128, PROD], F16, tag="G", name="G")
            if i == 0:
                # split per channel so work starts as each plane lands
                for ci in range(C):
                    nc.vector.tensor_sub(
                        tca[:, ci, :], c16all[:, ci, ef:PROD + ef],
                        c16all[:, ci, 0:PROD])
            else:
                nc.vector.tensor_sub(
                    tca[:], c16all[:, :, ef:PROD + ef], c16all[:, :, 0:PROD])
            if USE_PE_G:
                # DErf per channel so each channel's G matmuls overlap
                # the next channel's DErf (shortens the per-dir G-chain
                # latency; the PE idles during phase A anyway)
                for ch in range(C):
                    nc.scalar.activation(tca[:, ch, :], tca[:, ch, :],
                                         AF.Derivative_Erf, scale=SQS)
                    for r in range(R):
                        nc.tensor.matmul(
                            gps[:, r, :], wmat[ID_PLAIN],
                            tca[:, ch, r * W2C + COL0:r * W2C + COL0 + W],
                            start=(ch == 0), stop=(ch == C - 1),
                            skip_group_check=True,
                        )
                nc.scalar.activation(
                    G[:].rearrange("p (r w) -> p r w", w=W2C)[:, :, COL0:COL0 + W],
                    gps[:], AF.Copy)
            else:
                nc.scalar.activation(tca[:], tca[:],
                                     AF.Derivative_Erf, scale=SQS)
                nc.vector.tensor_add(G[:], tca[:, 0, :], tca[:, 1, :])
                nc.vector.tensor_add(G[:], G[:], tca[:, 2, :])

            F = Ft[i]
            nc.vector.tensor_mul(F, td, G[:])

            # pair-fused products: one op computes [B-term at 0, A-term
            # at +ef] (in1 = window pair {0, ef} of the mask / mc plane,
            # in0 = F broadcast); both fusion patterns measured at DVE 2x
            fm2 = fmp.tile([128, 2, PROD], F16, tag="fm2", name="fm2")
            nc.vector.tensor_mul(
                fm2[:], F.unsqueeze(1).broadcast_to([128, 2, PROD]),
                _window_ap(m16[:], 0, 2, PROD, step=ef))
            fhm, ftm = fm2[:, 0, :], fm2[:, 1, :]

            yz2 = yz_p.tile([128, 2, PROD], F16, tag="Y", name="yz2")
            nc.vector.tensor_mul(
                yz2[:], F.unsqueeze(1).broadcast_to([128, 2, PROD]),
                _window_ap(mc3[:, 0, :], 0, 2, PROD, step=ef))
            Z, Y = yz2[:, 0, :], yz2[:, 1, :]
            yz0.append((Y, Z))

            accum_dir(den, i, ftm, fhm, i == 0)
            if not USE_PE_G:
                accum_dir(num0, i, Y, Z, i == 0)

        accum_cen(den, m16[:])
        # 1/den = exp(-ln(den)); den>0 (products of exps, positive mask).
        nc.scalar.activation(lden[:], den[:], AF.Ln)
        nc.scalar.activation(r16[:], lden[:], AF.Exp, scale=-1.0)

        if USE_PE_G:
            # num0 accumulation (deferred so gps could share the PSUM ring)
            num0 = psum_p.tile([128, R, W], F32, tag="acc", name="num0",
                               bufs=2)
            num_sh = []
            for i in range(4):
                Y, Z = yz0[i]
                accum_dir(num0, i, Y, Z, i == 0, defer=num_sh)
            flush_sh(num0, num_sh)
        accum_cen(num0, mc3[:, 0, :])

        def _finals(num, ci, last=False):
            # two-row halves: copy/mul/store pipeline, shortens the tail;
            # the very last half runs in single-row quarters (it has
            # nothing left to overlap with except its own chain)
            chunks = [(0, 2), (2, 3), (3, 4)] if last else [(0, 2), (2, 4)]
            for k, (r0, r1) in enumerate(chunks):
                nr = r1 - r0
                n16 = fin_p.tile([128, nr, W], F16, tag=f"n16_{nr}",
                                 name="n16")
                nc.scalar.activation(n16[:], num[:, r0:r1, :], AF.Copy)
                o16 = fin_p.tile([128, nr, W], F16, tag=f"o16_{nr}",
                                 name="o16")
                nc.vector.tensor_mul(o16[:], n16[:], r16[:, r0:r1, :])
                (nc.sync if (ci + k) % 2 == 0 else nc.scalar).dma_start(
                    o_out[ci].rearrange("(p r) w -> p r w", r=R)[:, r0:r1, :],
                    o16[:]
                )

        _finals(num0, 0)

        # ---- phase B: channels 1,2 ----
        for ci in (1, 2):
            num = psum_p.tile([128, R, W], F32, tag="acc",
                              name=f"num{ci}", bufs=2)
            # dir0 pair product + fused Y/Z for the ef-consecutive dirs
            yz2 = yz_p.tile([128, 2, PROD], F16, tag="Y", name="yz2")
            nc.vector.tensor_mul(
                yz2[:], Ft[0].unsqueeze(1).broadcast_to([128, 2, PROD]),
                _window_ap(mc3[:, ci, :], 0, 2, PROD, step=1))
            Yf = yz_p.tile([128, 3, PROD], F16, tag="Yf", name="Yf", bufs=2)
            nc.vector.tensor_mul(
                Yf[:], Fall[:, 1:4, 0:PROD],
                _window_ap(mc3[:, ci, :], W2C - 1, 3, PROD))
            Zf = yz_p.tile([128, 3, PROD], F16, tag="Yf", name="Zf", bufs=2)
            nc.vector.tensor_mul(
                Zf[:], Fall[:, 1:4, 0:PROD],
                mc3[:, ci, 0:PROD].unsqueeze(1).broadcast_to([128, 3, PROD]))
            # dir0's -e tap is a pure column shift: pre-combine A+B on
            # the DVE (drops 4 B matmuls); emitted after Yf/Zf and
            # accumulated last so the add overlaps the dirs-1-3 matmuls.
            # Z's left pad supplies the zero boundary value (pads are
            # deterministically 0 via the gz memsets above).
            s0 = fp.tile([128, PROD], F16, tag="G", name="s0")
            nc.vector.tensor_add(s0[:, 1:PROD], yz2[:, 1, 1:PROD],
                                 yz2[:, 0, 0:PROD - 1])
            num_sh = []
            for i in (1, 2, 3, 0):
                if i == 0:
                    accum_dir(num, 0, s0[:], None, False, a_only=True)
                else:
                    accum_dir(num, i, Yf[:, SLOT[i] - 1, :],
                              Zf[:, SLOT[i] - 1, :], i == 1, defer=num_sh)
            flush_sh(num, num_sh)
            accum_cen(num, mc3[:, ci, :])
            _finals(num, ci, last=(ci == 2))


def _get_nc():
    if "nc" not in _BUILD_CACHE:
        _BUILD_CACHE["nc"] = _build_nc()
    return _BUILD_CACHE["nc"]


def _halo_planes(d, c, m):
    """[N,5,128,ALLOC] fp16: per-partition rows 4p..4p+4 x 520 cols with
    pad zeros baked in (2 left, 6 right, 24 tail slack)."""
    from numpy.lib.stride_tricks import as_strided
    stack = np.zeros((N_CORES, 5, H + 4, W2C), np.float16)
    for i in range(N_CORES):
        for k, arr in enumerate((d[i], c[i, 0], c[i, 1], c[i, 2], m[i])):
            stack[i, k, 0:H, COL0:COL0 + W] = arr
    s = stack.strides
    win = as_strided(
        stack,
        shape=(N_CORES, 5, 128, NROW, W2C),
        strides=(s[0], s[1], 4 * s[2], s[2], s[3]),
    )
    out = np.zeros((N_CORES, 5, 128, ALLOC), np.float16)
    out[:, :, :, 0:FLAT] = win.reshape(N_CORES, 5, 128, FLAT)
    return out


def _identities():
    eye = np.eye(128)
    sh = np.eye(128, k=1)
    return np.stack([
        eye, eye * W1, eye * W2, eye * WCEN, sh * W1, sh * W2, -eye,
    ]).astype(np.float16)


def _run(depth, color, mask, trace=False, **kw):
    nc = _get_nc()
    d = np.asarray(depth, dtype=np.float32).reshape(N_CORES, H, W)
    c = np.asarray(color, dtype=np.float32).reshape(N_CORES, C, H, W)
    m = np.asarray(mask, dtype=np.float32).reshape(N_CORES, H, W)
    x16 = _halo_planes(d, c, m)
    ids = _identities()
    in_maps = [{"x": x16[i], "ident": ids} for i in range(N_CORES)]
    res = run_bass_kernel_spmd(
        nc, in_maps, list(range(N_CORES)), trace=trace, **kw
    )
    out = np.stack([np.asarray(res.results[i]["out"]) for i in range(N_CORES)])
    return out.reshape(B, V, C, H, W).astype(np.float32), res


def kernel(depth, color, mask):
    out, _ = _run(depth, color, mask, trace=False)
    return out


# revision 65
# speedup vs baseline: 1.0107x; 1.0107x over previous
"""Bilateral filter (3x3, sigma=0.8) Trainium2 Bass kernel. v9

Sharding: fully data-parallel over the fused batch B*V = 8 -> one
(C=3,H=512,W=512) image per NeuronCore, 8 cores.

Per-core layout: H=512 rows split 4 rows/partition over 128 partitions.
Each partition holds 5 rows (4 data rows + 1 halo row below) x 520 cols
(2 left pad, 512 data, 6 right pad) flattened in the free dimension, so
every 3x3 tap at +e is a constant flat offset.  The host bakes all pad
zeros into the DMA'd planes, so no on-chip memsets gate the input DMA.

Math (unnormalized weights; per-pixel wd/wc normalizations cancel in
num/den; the 1e-7 eps is dropped, |effect| ~1e-4):
  For e in {(0,1),(1,0),(1,1),(1,-1)} (pair symmetry covers -e):
    g_e  = DErf(sqrt(S)(d(+e)-d))            # (2/sqrt(pi)) exp(-S dd^2)
    G_e  = sum_c DErf(sqrt(S)(c_c(+e)-c_c))
    F_e  = g_e * G_e                         # ws_e folded into PE weights
    den += ws_e*(F_e*m(+e) @0 + F_e*m @-e)   (+ center WCEN*m)
    num_c += ws_e*(F_e*mc_c(+e) @0 + F_e*mc_c @-e) (+ center WCEN*mc_c)
  with mc_c = m*c_c precomputed once.  The DErf constant (2/sqrt(pi))^2
  is uniform across taps once the center is scaled by 4/pi, so it
  cancels in num/den.

Engine split (the DVE is the bottleneck; the HAM clock governor is a
POWER budget, so junk/offload work on other engines slows everything —
GpSimd compute measured ~20% global clock loss):
 - DVE: diffs/products, all fused where flat offsets allow: the three
   ef-consecutive dirs (519,520,521) share one op via overlapping-window
   APs; {A,B}-side product pairs share one op via a window-pair AP with
   the F field broadcast (stride-0); all verified to keep DVE 2x mode.
 - ACT: DErf/Ln/Exp/PSUM copies.
 - PE: G channel-sum (scaled-identity matmuls into PSUM) + all 9-tap
   accumulations of den/num_c; per-row FD-512 matmuls (PSUM bank cap);
   -e taps crossing a partition boundary use shift matrices
   sh=eye(k=1)*ws, deferred per accumulation for Ldweights dedup.
Finals run in 2-row halves (PSUM copy / 1/den multiply / store DMA
pipeline; the last channel's second half in 1-row quarters) to shorten
the serial tail.
"""

import math
import numpy as np
import sys

if "/opt/trn_rl_repo" not in sys.path:
    sys.path.insert(0, "/opt/trn_rl_repo")

import concourse.bass as bass
import concourse.tile as tile
from concourse import mybir
from concourse.bass_utils import run_bass_kernel_spmd

# ---- problem constants (hardcoded per spec) ----
B, V, C, H, W = 2, 4, 3, 512, 512
N_CORES = 8
KS = 3
SIG = 0.3 * ((KS - 1) * 0.5 - 1) + 0.8           # 0.8
S = 1.0 / (2.0 * SIG * SIG)                       # 0.78125

# spatial gaussian, normalized
_xs = np.arange(KS, dtype=np.float64)
_gx, _gy = np.meshgrid(_xs, _xs, indexing="xy")
_w = np.exp(-(((_gx - 1) ** 2 + (_gy - 1) ** 2)) * S)
_w = _w / _w.sum()
W0 = float(_w[1, 1])   # center
W1 = float(_w[0, 1])   # edge-adjacent
W2 = float(_w[0, 0])   # diagonal

# layout constants
R = 4                  # data rows per partition
W2C = 520              # row stride (2 left pad + 512 data + 6 right pad)
NROW = 5               # rows per partition incl. bottom halo
FLAT = NROW * W2C      # 2600
ALLOC = FLAT + 24      # slack so reads at +521 from flat 2079 stay in-bounds
PROD = R * W2C         # 2080: field/product grid (4 rows, all cols)
COL0 = 2               # first data col

# (er, ec, flat offset, spatial weight index)
ES = [(0, 1, 1, 0), (1, 0, W2C, 0), (1, 1, W2C + 1, 1), (1, -1, W2C - 1, 1)]
SQS = math.sqrt(S)          # DErf(SQS*x) = 2/sqrt(pi) * exp(-S x^2)
PHI2 = 4.0 / math.pi        # (2/sqrt(pi))^2, folded into the center tap
WCEN = 3.0 * W0 * PHI2

# ---- tuning knobs ----
USE_PE_G = True             # G channel-sum via PE matmuls + ACT copy
N_WARM = 0                  # PE warm-up matmuls (fills pre-field idle)

# dirs sorted by flat offset ef (519, 520, 521 are consecutive, so the
# phase-B products for those three dirs fuse into one DVE op via an
# overlapping-window AP); SLOT maps dir index -> slot in the Fall tile
SLOT = {0: 0, 3: 1, 1: 2, 2: 3}   # dir -> slot in the Fall tile


def _window_ap(ap2d, off, count, length, step=1):
    """[p, count, length] view of a flat [p, N] AP with overlapping
    windows starting at off, off+step, ..., off+(count-1)*step."""
    w = ap2d[:, off:off + length].unsqueeze(1).copy()
    v = w.ap
    v[1] = [step, count]
    w.ap = v
    return w

F16 = mybir.dt.float16
F32 = mybir.dt.float32
AF = mybir.ActivationFunctionType
ALU = mybir.AluOpType

# weight-matrix slots in the idents tile
ID_PLAIN, ID_W1, ID_W2, ID_CEN, ID_SH1, ID_SH2, ID_NEG = range(7)


# ---- walrus single-wait workaround ----------------------------------------
# This container's walrus accepts only ONE sync_info.on_wait per instruction;
# Tile emits multi-wait instructions. Hoist all but the last wait onto
# injected single-wait NoOps just before the original.
import orjson as _orjson

_SCRATCH = "wsplit_scratch"


def _mk_nop(name, engine, wait):
    return {"name": name, "engine": engine, "ins": [], "outs": [],
            "opcode": "NoOp",
            "sync_info": {"on_wait": [wait], "on_update": []}}


def _ldw_sig(ins):
    aps = ins.get("ins") or []
    if not aps:
        return None
    a = aps[0]
    return (a.get("memref"), a.get("offset"), str(a.get("ap")), a.get("dtype"))


def _dedup_ldweights(m):
    """NoOp-ify PE Ldweights whose weights are already loaded (same static
    source AP as the previous Ldweights, sourced from the idents tile).
    Sync info is preserved on the NoOp."""
    for f in m.get("functions", []):
        for bb in f.get("blocks", []):
            last = None
            for ins in bb.get("instructions", []):
                if ins.get("opcode") != "Ldweights":
                    continue
                sig = _ldw_sig(ins)
                if (sig is not None and sig == last
                        and sig[0] and "idents" in sig[0]):
                    ins["opcode"] = "NoOp"
                    ins["ins"] = []
                    ins["outs"] = []
                else:
                    last = sig
    return m


def _split_multiwaits(bir_bytes):
    m = _orjson.loads(bir_bytes)
    _dedup_ldweights(m)
    for f in m.get("functions", []):
        for bb in f.get("blocks", []):
            out = []
            for ins in bb.get("instructions", []):
                si = ins.get("sync_info")
                waits = (si or {}).get("on_wait") or []
                if len(waits) > 1:
                    for k, w in enumerate(waits[:-1]):
                        nm = f"{ins['name']}-wsplit{k}"
                        out.append(_mk_nop(nm, ins["engine"], w))
                    si["on_wait"] = [waits[-1]]
                out.append(ins)
            bb["instructions"] = out
    return _orjson.dumps(m)


_BUILD_CACHE = {}


def _build_nc():
    nc = bass.Bass()
    x_in = nc.declare_dram_parameter("x", [5, 128, ALLOC], F16, isOutput=False)
    id_in = nc.declare_dram_parameter("ident", [7, 128, 128], F16, isOutput=False)
    o_out = nc.declare_dram_parameter("out", [C, H, W], F16, isOutput=True)
    nc.dram_tensor(_SCRATCH, [4], F32)

    with tile.TileContext(nc) as tc:
        _emit(nc, tc, x_in, id_in, o_out)

    orig_to_json = nc.to_json_bytes
    nc.to_json_bytes = lambda: _split_multiwaits(orig_to_json())
    return nc


def _emit(nc, tc, x_in, id_in, o_out):
    from contextlib import ExitStack
    ctx = ExitStack()
    with ctx:
        persist = ctx.enter_context(tc.tile_pool(name="persist", bufs=1))
        tdp = ctx.enter_context(tc.tile_pool(name="tdp", bufs=2))
        tcap = ctx.enter_context(tc.tile_pool(name="tcap", bufs=2))
        fp = ctx.enter_context(tc.tile_pool(name="fp", bufs=2))
        fmp = ctx.enter_context(tc.tile_pool(name="fmp", bufs=2))
        yz_p = ctx.enter_context(tc.tile_pool(name="yz", bufs=4))
        fin_p = ctx.enter_context(tc.tile_pool(name="fin", bufs=2))
        psum_p = ctx.enter_context(
            tc.tile_pool(name="psum", bufs=1, space=bass.MemorySpace.PSUM)
        )

        # ---- persistent fp16 planes / fields ----
        d16 = persist.tile([128, ALLOC], F16, tag="d16", name="d16")
        m16 = persist.tile([128, ALLOC], F16, tag="m16", name="m16")
        c16all = persist.tile([128, C, ALLOC], F16, tag="c16all", name="c16all")
        c16 = [c16all[:, i, :] for i in range(C)]
        mc3 = persist.tile([128, C, ALLOC], F16, tag="mc3", name="mc3")
        idents = persist.tile([128, 7, 128], F16, tag="idents", name="idents")
        wmat = [idents[:, j, :] for j in range(7)]
        Fall = persist.tile([128, 4, PROD], F16, tag="Fall", name="Fall")
        Ft = [Fall[:, SLOT[i], :] for i in range(4)]
        r16 = persist.tile([128, R, W], F16, tag="r16", name="r16")
        lden = persist.tile([128, R, W], F32, tag="lden", name="lden")

        # ---- load planes (fully pre-padded host-side; no memsets) ----
        # order: d first (fields), then c0..c2 (tca), m, idents early for
        # PE warm-up.  Two HWDGE queues: sync + scalar.
        nc.scalar.dma_start(idents[:], id_in.rearrange("j p c -> p j c"))
        nc.sync.dma_start(d16[:], x_in[0])
        nc.scalar.dma_start(c16all[:, 0, :], x_in[1])
        nc.sync.dma_start(c16all[:, 1, :], x_in[2])
        nc.scalar.dma_start(c16all[:, 2, :], x_in[3])
        nc.sync.dma_start(m16[:], x_in[4])

        den = psum_p.tile([128, R, W], F32, tag="acc", name="den", bufs=2)

        # NOTE: the HAM clock governor enforces a POWER budget — dummy
        # "heater" work makes the whole kernel clock LOWER (measured
        # +30us).  Keep extra activity minimal; a short PE warm-up only.
        for k in range(N_WARM):
            nc.tensor.matmul(
                den[:, 0, 0:W], wmat[ID_PLAIN], idents[:, 0:4, :],
                start=True, stop=True, skip_group_check=True,
            )

        # mc_c = m * c_c on the full 5-row grid (halos included); one
        # fused op over all 3 channels with m broadcast (stride-0 dim)
        nc.vector.tensor_mul(
            mc3[:, :, :], c16all[:, :, :],
            m16[:].unsqueeze(1).broadcast_to([128, C, ALLOC]))

        def mm(acc, wi, rhs_flat, off, row, start=False, stop=False,
               n=W, ocol=0):
            nc.tensor.matmul(
                acc[:, row, ocol:ocol + n], wmat[wi],
                rhs_flat[:, off:off + n],
                start=start, stop=stop,
            )

        def accum_dir(acc, i, a_t, b_t, first, defer=None, a_only=False):
            """acc += ws_e*(A-term at 0) + ws_e*(B-term at -e) for dir i.
            B windows skip the boundary column (where the halo tap is
            zero); the cross-partition row-0 B tap uses a shift matrix.
            If `defer` is a list the shift matmul is queued there instead
            (flush with flush_sh) so consecutive dirs keep the same PE
            weights loaded (Ldweights dedup).  a_only skips the B block
            (used when a_t already holds the A+B combined plane)."""
            er, ec, ef, iw = ES[i]
            widn = ID_W1 if iw == 0 else ID_W2
            wsh = ID_SH1 if iw == 0 else ID_SH2
            for r in range(R):
                mm(acc, widn, a_t, r * W2C + COL0, r, start=first)
            if a_only:
                return
            n = W - abs(ec)
            ocol = max(0, ec)
            icol = COL0 + max(0, -ec)
            if er == 0:
                for r in range(R):
                    mm(acc, widn, b_t, r * W2C + icol, r, n=n, ocol=ocol)
            else:
                for r in range(1, R):
                    mm(acc, widn, b_t, (r - 1) * W2C + icol, r, n=n, ocol=ocol)
                if defer is None:
                    mm(acc, wsh, b_t, 3 * W2C + icol, 0, n=n, ocol=ocol)
                else:
                    defer.append((wsh, b_t, 3 * W2C + icol, n, ocol))

        def flush_sh(acc, deferred):
            deferred.sort(key=lambda t: t[0])
            for wsh, b_t, off, n, ocol in deferred:
                mm(acc, wsh, b_t, off, 0, n=n, ocol=ocol)
            deferred.clear()

        def accum_cen(acc, cen_t, stop=True):
            for r in range(R):
                mm(acc, ID_CEN, cen_t, r * W2C + COL0, r, stop=(stop and r == R - 1))

        # ---- phase A: per-dir fields + den accumulation; ch0 products ----
        # USE_PE_G: G channel-sum runs on the PE into a PSUM tile (gps),
        # copied back to fp16 SBUF by the scalar engine; this takes 8
        # tensor_adds off the DVE but occupies 4 PSUM banks, deferring
        # num0 accumulation to phase B.  Otherwise G sums on the DVE and
        # den+num0 accumulate interleaved in phase A (balanced PE phases).
        if USE_PE_G:
            gps = psum_p.tile([128, R, W], F32, tag="acc", name="gps",
                              bufs=2)
        else:
            num0 = psum_p.tile([128, R, W], F32, tag="acc", name="num0",
                               bufs=2)

        # td fields: dir0 alone (earliest), dirs 3,1,2 fused in one op via
        # the overlapping-window AP (ef = 519,520,521) with d16 broadcast
        td0 = tdp.tile([128, PROD], F16, tag="td", name="td0", bufs=1)
        nc.vector.tensor_sub(td0[:], d16[:, 1:PROD + 1], d16[:, 0:PROD])
        nc.scalar.activation(td0[:], td0[:], AF.Derivative_Erf, scale=SQS)
        tdf = tdp.tile([128, 3, PROD], F16, tag="tdf", name="tdf", bufs=1)
        nc.vector.tensor_sub(
            tdf[:], _window_ap(d16[:], W2C - 1, 3, PROD),
            d16[:, 0:PROD].unsqueeze(1).broadcast_to([128, 3, PROD]))
        nc.scalar.activation(tdf[:], tdf[:], AF.Derivative_Erf, scale=SQS)

        yz0 = []
        for i, (er, ec, ef, iw) in enumerate(ES):
            td = td0[:] if i == 0 else tdf[:, SLOT[i] - 1, :]

            tca = tcap.tile([128, C, PROD], F16, tag="tca", name="tca")
            G = fp.tile([128, PROD], F16, tag="G", name="G")
            if i == 0:
                # split per channel so work starts as each plane lands
                for ci in range(C):
                    nc.vector.tensor_sub(
                        tca[:, ci, :], c16all[:, ci, ef:PROD + ef],
                        c16all[:, ci, 0:PROD])
            else:
                nc.vector.tensor_sub(
                    tca[:], c16all[:, :, ef:PROD + ef], c16all[:, :, 0:PROD])
            if USE_PE_G:
                # DErf per channel so each channel's G matmuls overlap
                # the next channel's DErf (shortens the per-dir G-chain
                # latency; the PE idles during phase A anyway)
                for ch in range(C):
                    nc.scalar.activation(tca[:, ch, :], tca[:, ch, :],
                                         AF.Derivative_Erf, scale=SQS)
                    for r in range(R):
                        nc.tensor.matmul(
                            gps[:, r, :], wmat[ID_PLAIN],
                            tca[:, ch, r * W2C + COL0:r * W2C + COL0 + W],
                            start=(ch == 0), stop=(ch == C - 1),
                            skip_group_check=True,
                        )
                nc.scalar.activation(
                    G[:].rearrange("p (r w) -> p r w", w=W2C)[:, :, COL0:COL0 + W],
                    gps[:], AF.Copy)
            else:
                nc.scalar.activation(tca[:], tca[:],
                                     AF.Derivative_Erf, scale=SQS)
                nc.vector.tensor_add(G[:], tca[:, 0, :], tca[:, 1, :])
                nc.vector.tensor_add(G[:], G[:], tca[:, 2, :])

            F = Ft[i]
            nc.vector.tensor_mul(F, td, G[:])

            # pair-fused products: one op computes [B-term at 0, A-term
            # at +ef] (in1 = window pair {0, ef} of the mask / mc plane,
            # in0 = F broadcast); both fusion patterns measured at DVE 2x
            fm2 = fmp.tile([128, 2, PROD], F16, tag="fm2", name="fm2")
            nc.vector.tensor_mul(
                fm2[:], F.unsqueeze(1).broadcast_to([128, 2, PROD]),
                _window_ap(m16[:], 0, 2, PROD, step=ef))
            fhm, ftm = fm2[:, 0, :], fm2[:, 1, :]

            yz2 = yz_p.tile([128, 2, PROD], F16, tag="Y", name="yz2")
            nc.vector.tensor_mul(
                yz2[:], F.unsqueeze(1).broadcast_to([128, 2, PROD]),
                _window_ap(mc3[:, 0, :], 0, 2, PROD, step=ef))
            Z, Y = yz2[:, 0, :], yz2[:, 1, :]
            yz0.append((Y, Z))

            accum_dir(den, i, ftm, fhm, i == 0)
            if not USE_PE_G:
                accum_dir(num0, i, Y, Z, i == 0)

        accum_cen(den, m16[:])
        # 1/den = exp(-ln(den)); den>0 (products of exps, positive mask).
        nc.scalar.activation(lden[:], den[:], AF.Ln)
        nc.scalar.activation(r16[:], lden[:], AF.Exp, scale=-1.0)

        if USE_PE_G:
            # num0 accumulation (deferred so gps could share the PSUM ring)
            num0 = psum_p.tile([128, R, W], F32, tag="acc", name="num0",
                               bufs=2)
            num_sh = []
            for i in range(4):
                Y, Z = yz0[i]
                accum_dir(num0, i, Y, Z, i == 0, defer=num_sh)
            flush_sh(num0, num_sh)
        accum_cen(num0, mc3[:, 0, :])

        def _finals(num, ci, last=False):
            # two-row halves: copy/mul/store pipeline, shortens the tail;
            # the very last half runs in single-row quarters (it has
            # nothing left to overlap with except its own chain)
            chunks = [(0, 2), (2, 3), (3, 4)] if last else [(0, 2), (2, 4)]
            for k, (r0, r1) in enumerate(chunks):
                nr = r1 - r0
                n16 = fin_p.tile([128, nr, W], F16, tag=f"n16_{nr}",
                                 name="n16")
                nc.scalar.activation(n16[:], num[:, r0:r1, :], AF.Copy)
                o16 = fin_p.tile([128, nr, W], F16, tag=f"o16_{nr}",
                                 name="o16")
                nc.vector.tensor_mul(o16[:], n16[:], r16[:, r0:r1, :])
                (nc.sync if (ci + k) % 2 == 0 else nc.scalar).dma_start(
                    o_out[ci].rearrange("(p r) w -> p r w", r=R)[:, r0:r1, :],
                    o16[:]
                )

        _finals(num0, 0)

        # ---- phase B: channels 1,2 ----
        for ci in (1, 2):
            num = psum_p.tile([128, R, W], F32, tag="acc",
                              name=f"num{ci}", bufs=2)
            # dir0 pair product + fused Y/Z for the ef-consecutive dirs
            yz2 = yz_p.tile([128, 2, PROD], F16, tag="Y", name="yz2")
            nc.vector.tensor_mul(
                yz2[:], Ft[0].unsqueeze(1).broadcast_to([128, 2, PROD]),
                _window_ap(mc3[:, ci, :], 0, 2, PROD, step=1))
            Z0, Y0 = yz2[:, 0, :], yz2[:, 1, :]
            Yf = yz_p.tile([128, 3, PROD], F16, tag="Yf", name="Yf", bufs=2)
            nc.vector.tensor_mul(
                Yf[:], Fall[:, 1:4, 0:PROD],
                _window_ap(mc3[:, ci, :], W2C - 1, 3, PROD))
            Zf = yz_p.tile([128, 3, PROD], F16, tag="Yf", name="Zf", bufs=2)
            nc.vector.tensor_mul(
                Zf[:], Fall[:, 1:4, 0:PROD],
                mc3[:, ci, 0:PROD].unsqueeze(1).broadcast_to([128, 3, PROD]))
            num_sh = []
            for i in range(4):
                Yap = Y0 if i == 0 else Yf[:, SLOT[i] - 1, :]
                Zap = Z0 if i == 0 else Zf[:, SLOT[i] - 1, :]
                accum_dir(num, i, Yap, Zap, i == 0, defer=num_sh)
            flush_sh(num, num_sh)
            accum_cen(num, mc3[:, ci, :])
            _finals(num, ci, last=(ci == 2))


def _get_nc():
    if "nc" not in _BUILD_CACHE:
        _BUILD_CACHE["nc"] = _build_nc()
    return _BUILD_CACHE["nc"]


def _halo_planes(d, c, m):
    """[N,5,128,ALLOC] fp16: per-partition rows 4p..4p+4 x 520 cols with
    pad zeros baked in (2 left, 6 right, 24 tail slack)."""
    from numpy.lib.stride_tricks import as_strided
    stack = np.zeros((N_CORES, 5, H + 4, W2C), np.float16)
    for i in range(N_CORES):
        for k, arr in enumerate((d[i], c[i, 0], c[i, 1], c[i, 2], m[i])):
            stack[i, k, 0:H, COL0:COL0 + W] = arr
    s = stack.strides
    win = as_strided(
        stack,
        shape=(N_CORES, 5, 128, NROW, W2C),
        strides=(s[0], s[1], 4 * s[2], s[2], s[3]),
    )
    out = np.zeros((N_CORES, 5, 128, ALLOC), np.float16)
    out[:, :, :, 0:FLAT] = win.reshape(N_CORES, 5, 128, FLAT)
    return out


def _identities():
    eye = np.eye(128)
    sh = np.eye(128, k=1)
    return np.stack([
        eye, eye * W1, eye * W2, eye * WCEN, sh * W1, sh * W2, -eye,
    ]).astype(np.float16)


def _run(depth, color, mask, trace=False, **kw):
    nc = _get_nc()
    d = np.asarray(depth, dtype=np.float32).reshape(N_CORES, H, W)
    c = np.asarray(color, dtype=np.float32).reshape(N_CORES, C, H, W)
    m = np.asarray(mask, dtype=np.float32).reshape(N_CORES, H, W)
    x16 = _halo_planes(d, c, m)
    ids = _identities()
    in_maps = [{"x": x16[i], "ident": ids} for i in range(N_CORES)]
    res = run_bass_kernel_spmd(
        nc, in_maps, list(range(N_CORES)), trace=trace, **kw
    )
    out = np.stack([np.asarray(res.results[i]["out"]) for i in range(N_CORES)])
    return out.reshape(B, V, C, H, W).astype(np.float32), res


def kernel(depth, color, mask):
    out, _ = _run(depth, color, mask, trace=False)
    return out
